# revision 10
# baseline (speedup 1.0000x reference)
"""Distributed Trainium2 Bass kernel for causal GQA attention block.

Problem (hardcoded): x [4, 2048, 1024] f32; wq [1024, 1024]; wk/wv [1024, 256];
wo [1024, 1024]. 16 q-heads, 4 kv-heads, head_dim 64, rms-norm on q/k (no
weight), rope (base 10000), q gain 1.5, causal SDPA, out-proj.

Sharding over 8 cores: core i -> batch b = i//2, head-half p = i%2
(q-heads 8p..8p+7, kv-heads 2p, 2p+1 -- KV groups intact). Each core computes
its 8 heads' attention output O^T (feature-major), pairs AllGather O^T, and
each core computes a disjoint 512-column slice of the out-projection.

On-chip layouts are feature-major ("transposed"): X^T, Q^T, K^T so the PE
contracts over partitions; V is token-major with a ones column appended so the
PV matmul also produces softmax row-sums (normalization happens on O^T).
"""
import sys

sys.path.insert(0, "/opt/trn_rl_repo")

import numpy as np
import ml_dtypes

import concourse.bacc as bacc
import concourse.mybir as mybir
import concourse.tile as tile
from concourse.bass_utils import run_bass_kernel_spmd

F32 = mybir.dt.float32
BF16 = mybir.dt.bfloat16
AF = mybir.ActivationFunctionType

N = 2048          # tokens
C = 1024          # model dim
DQ = 512          # local q out-features (8 heads x 64)
DKV = 128         # local kv out-features (2 kv heads x 64)
D = 64            # head dim
NCC = C // 128    # 8 contraction chunks
NQT = 4           # q tiles of 512
NTC = N // 128    # 16 token chunks
QK_GAIN = 1.5
ROPE_BASE = 10000.0
EXP_SCALE = QK_GAIN / np.sqrt(D).item()  # folded gain * 1/sqrt(D) = 0.1875
EPS = float(np.finfo(np.float32).eps)


def _host_tables():
    inv_freq = (1.0 / (ROPE_BASE ** (np.arange(0, D, 2, dtype=np.float64) / D)))  # [32]
    t = np.arange(N, dtype=np.float64)
    ang = np.outer(inv_freq, t)  # [32, N]
    cos32 = np.cos(ang)
    sin32 = np.sin(ang)
    cosT = np.tile(cos32, (4, 1)).astype(np.float32)  # [128, N]
    sinTs = np.concatenate([-sin32, sin32, -sin32, sin32], axis=0).astype(np.float32)
    q = np.arange(128)
    trimask = (q[None, :] >= q[:, None]).astype(np.float32)  # keep q >= k
    ones2 = np.zeros((128, 2), np.float32)
    ones2[0:64, 0] = 1.0
    ones2[64:128, 1] = 1.0
    expA = np.zeros((2, 64), np.float32)
    expA[0, :] = 1.0
    expB = np.zeros((2, 64), np.float32)
    expB[1, :] = 1.0
    bf = ml_dtypes.bfloat16
    return {
        "cosT": cosT.astype(bf),
        "sinTs": sinTs.astype(bf),
        "trimask": trimask.astype(bf),
        "ones2": ones2.astype(bf),
        "expA": expA.astype(bf),
        "expB": expB.astype(bf),
    }


def build():
    nc = bacc.Bacc(None, target_bir_lowering=False, num_devices=8)

    x_ext = nc.declare_dram_parameter("x", [N, C], F32, isOutput=False)
    wq_ext = nc.declare_dram_parameter("wq", [C, DQ], F32, isOutput=False)
    wk_ext = nc.declare_dram_parameter("wk", [C, DKV], F32, isOutput=False)
    wv_ext = nc.declare_dram_parameter("wv", [C, DKV], F32, isOutput=False)
    wo_ext = nc.declare_dram_parameter("wo", [C, DQ], F32, isOutput=False)
    out_ext = nc.declare_dram_parameter("out", [N, DQ], F32, isOutput=True)

    tabs = _host_tables()
    cosT_d = nc.inline_tensor(tabs["cosT"], name="cosT_d")
    sinTs_d = nc.inline_tensor(tabs["sinTs"], name="sinTs_d")
    trimask_d = nc.inline_tensor(tabs["trimask"], name="trimask_d")
    ones2_d = nc.inline_tensor(tabs["ones2"], name="ones2_d")
    expA_d = nc.inline_tensor(tabs["expA"], name="expA_d")
    expB_d = nc.inline_tensor(tabs["expB"], name="expB_d")

    with tile.TileContext(nc) as tc:
        with (
            tc.tile_pool(name="dram", bufs=1, space="DRAM") as dram,
            tc.tile_pool(name="persist", bufs=1) as ps,
            tc.tile_pool(name="work", bufs=3) as wk,
        ):
            # ---- persistent SBUF tensors ----
            xT = ps.tile([128, NCC, N], BF16, name="xT")          # X^T chunks
            wq_sb = ps.tile([128, NCC, DQ], BF16, name="wq_sb")
            wk_sb = ps.tile([128, NCC, DKV], BF16, name="wk_sb")
            wv_sb = ps.tile([128, NCC, DKV], BF16, name="wv_sb")
            wo_sb = ps.tile([128, NCC, DQ], BF16, name="wo_sb")
            cosT = ps.tile([128, N], BF16, name="cosT")
            sinTs = ps.tile([128, N], BF16, name="sinTs")
            trimask = ps.tile([128, 128], BF16, name="trimask")
            ones2 = ps.tile([128, 2], BF16, name="ones2")
            expA = ps.tile([2, 64], BF16, name="expA")
            expB = ps.tile([2, 64], BF16, name="expB")
            eps_sb = ps.tile([128, 1], F32, name="eps_sb")
            qTf = ps.tile([128, 4, N], BF16, name="qTf")          # final Q^T
            kTdA = ps.tile([128, N], BF16, name="kTdA")           # kv head A dup'd
            kTdB = ps.tile([128, N], BF16, name="kTdB")
            v_sb = ps.tile([128, NTC, 130], BF16, name="v_sb")    # [V_A|1|V_B|1]
            oT = ps.tile([128, 4, N], BF16, name="oT")            # own O^T (normed)
            oT_all = ps.tile([128, 8, N], BF16, name="oT_all")    # gathered pair

            # ---- phase A: stage inputs ----
            x_bf = dram.tile([N, C], BF16)
            wq_bf = dram.tile([C, DQ], BF16)
            wk_bf = dram.tile([C, DKV], BF16)
            wv_bf = dram.tile([C, DKV], BF16)
            wo_bf = dram.tile([C, DQ], BF16)
            # f32 -> bf16 cast DMAs (SWDGE)
            for tcix in range(NTC):
                nc.gpsimd.dma_start(
                    out=x_bf[tcix * 128:(tcix + 1) * 128, :],
                    in_=x_ext[tcix * 128:(tcix + 1) * 128, :],
                )
            nc.gpsimd.dma_start(out=wq_bf[:], in_=wq_ext[:])
            nc.gpsimd.dma_start(out=wk_bf[:], in_=wk_ext[:])
            nc.gpsimd.dma_start(out=wv_bf[:], in_=wv_ext[:])
            nc.gpsimd.dma_start(out=wo_bf[:], in_=wo_ext[:])
            # transpose-DMAs: xT[p, cc, n] = x_bf[n, cc*128+p]
            for cc in range(NCC):
                nc.sync.dma_start(
                    out=xT[:, cc, :], in_=x_bf[:, cc * 128:(cc + 1) * 128],
                    transpose=True,
                )
            # weight loads [(a p) j -> p a j]
            nc.sync.dma_start(out=wq_sb[:], in_=wq_bf.rearrange("(a p) j -> p a j", p=128))
            nc.sync.dma_start(out=wk_sb[:], in_=wk_bf.rearrange("(a p) j -> p a j", p=128))
            nc.sync.dma_start(out=wv_sb[:], in_=wv_bf.rearrange("(a p) j -> p a j", p=128))
            nc.sync.dma_start(out=wo_sb[:], in_=wo_bf.rearrange("(a p) j -> p a j", p=128))
            # constants
            nc.sync.dma_start(out=cosT[:], in_=cosT_d[:])
            nc.sync.dma_start(out=sinTs[:], in_=sinTs_d[:])
            nc.sync.dma_start(out=trimask[:], in_=trimask_d[:])
            nc.sync.dma_start(out=ones2[:], in_=ones2_d[:])
            nc.sync.dma_start(out=expA[:], in_=expA_d[:])
            nc.sync.dma_start(out=expB[:], in_=expB_d[:])
            nc.gpsimd.memset(eps_sb[:], EPS)
            nc.gpsimd.memset(v_sb[:, :, 64:65], 1.0)
            nc.gpsimd.memset(v_sb[:, :, 129:130], 1.0)

            # ---- phase B: projections ----
            with (
                tc.tile_pool(name="bc_psum", bufs=2, space="PSUM") as bp,
                tc.tile_pool(name="bc_sbuf", bufs=3) as bs,
            ):
                qT_raw = bs.tile([128, 4, N], BF16, name="qT_raw", bufs=1)
                kT_raw = bs.tile([128, N], BF16, name="kT_raw", bufs=1)
                for m in range(4):
                    for qt in range(NQT):
                        pp = bp.tile([128, 512], F32, tag="proj")
                        for cc in range(NCC):
                            nc.tensor.matmul(
                                pp[:], wq_sb[:, cc, m * 128:(m + 1) * 128],
                                xT[:, cc, qt * 512:(qt + 1) * 512],
                                start=(cc == 0), stop=(cc == NCC - 1),
                            )
                        nc.vector.tensor_copy(qT_raw[:, m, qt * 512:(qt + 1) * 512], pp[:])
                for qt in range(NQT):
                    pp = bp.tile([128, 512], F32, tag="proj")
                    for cc in range(NCC):
                        nc.tensor.matmul(
                            pp[:], wk_sb[:, cc, :], xT[:, cc, qt * 512:(qt + 1) * 512],
                            start=(cc == 0), stop=(cc == NCC - 1),
                        )
                    nc.vector.tensor_copy(kT_raw[:, qt * 512:(qt + 1) * 512], pp[:])
                for tcix in range(NTC):
                    pv = bp.tile([128, 128], F32, tag="vproj")
                    for cc in range(NCC):
                        nc.tensor.matmul(
                            pv[:], xT[:, cc, tcix * 128:(tcix + 1) * 128],
                            wv_sb[:, cc, :],
                            start=(cc == 0), stop=(cc == NCC - 1),
                        )
                    nc.vector.tensor_copy(v_sb[:, tcix, 0:64], pv[:, 0:64])
                    nc.vector.tensor_copy(v_sb[:, tcix, 65:129], pv[:, 64:128])

                # ---- phase C: rms-norm + rope ----
                chunks = [(qT_raw[:, m, :], qTf[:, m, :]) for m in range(4)]
                chunks.append((kT_raw[:], kTdA[:]))  # kT final written to kTdA first
                for ci, (src, dst) in enumerate(chunks):
                    sq = bs.tile([128, N], BF16, tag="sq", bufs=2)
                    nc.vector.tensor_mul(sq[:], src, src)
                    lnv = bs.tile([2, N], F32, tag="lnv", bufs=2)
                    for qt in range(NQT):
                        msp = bp.tile([2, 512], F32, tag="ms")
                        nc.tensor.matmul(
                            msp[:], ones2[:],
                            sq[:, qt * 512:(qt + 1) * 512], start=True, stop=True,
                        )
                        nc.scalar.activation(
                            lnv[:, qt * 512:(qt + 1) * 512], msp[:],
                            AF.Ln, bias=eps_sb[0:2, :], scale=1.0 / D,
                        )
                    rr2 = bs.tile([2, N], BF16, tag="rr2", bufs=2)
                    nc.scalar.activation(rr2[:], lnv[:], AF.Exp, scale=-0.5)
                    rot = bs.tile([128, N], BF16, tag="rot", bufs=2)
                    nc.vector.tensor_copy(rot[0:32, :], src[32:64, :])
                    nc.vector.tensor_copy(rot[32:64, :], src[0:32, :])
                    nc.vector.tensor_copy(rot[64:96, :], src[96:128, :])
                    nc.vector.tensor_copy(rot[96:128, :], src[64:96, :])
                    t1 = bs.tile([128, N], BF16, tag="t1", bufs=2)
                    nc.vector.tensor_mul(t1[:], src, cosT[:])
                    nc.vector.tensor_mul(rot[:], rot[:], sinTs[:])
                    nc.vector.tensor_add(t1[:], t1[:], rot[:])
                    # apply r (per head, per token) via expander broadcast
                    for qt in range(NQT):
                        rbp = bp.tile([128, 512], F32, tag="rbp")
                        # rows 0:64 <- rr2[0], rows 64:128 <- rr2[1]
                        nc.tensor.matmul(
                            rbp[0:64, :], expA[:], rr2[:, qt * 512:(qt + 1) * 512],
                            start=True, stop=True,
                        )
                        nc.tensor.matmul(
                            rbp[64:128, :], expB[:], rr2[:, qt * 512:(qt + 1) * 512],
                            start=True, stop=True,
                        )
                        nc.vector.tensor_mul(
                            dst[:, qt * 512:(qt + 1) * 512],
                            t1[:, qt * 512:(qt + 1) * 512], rbp[:],
                        )
                # duplicate kv halves: kTdA currently holds full kT (A rows 0:64, B 64:128)
                nc.vector.tensor_copy(kTdB[0:64, :], kTdA[64:128, :])
                nc.vector.tensor_copy(kTdB[64:128, :], kTdA[64:128, :])
                nc.vector.tensor_copy(kTdA[64:128, :], kTdA[0:64, :])

            # ---- phase D: attention ----
            with (
                tc.tile_pool(name="at_psum", bufs=1, space="PSUM") as ap_,
                tc.tile_pool(name="at_sbuf", bufs=3) as as_,
            ):
                for m in range(4):
                    kT = kTdA if m < 2 else kTdB
                    vslot = 0 if m < 2 else 65
                    for j in range(NQT):
                        oa = ap_.tile([65, 512], F32, tag="oa", bufs=1)
                        ob = ap_.tile([65, 512], F32, tag="ob", bufs=1)
                        nkc = 4 * (j + 1)
                        for kc in range(nkc):
                            i = kc - 4 * j
                            off = max(0, 128 * i)
                            w = 512 - off
                            q0 = 512 * j + off
                            sA = ap_.tile([128, 512], F32, tag="sA", bufs=2)
                            sB = ap_.tile([128, 512], F32, tag="sB", bufs=2)
                            nc.tensor.matmul(
                                sA[:, 0:w], kT[0:64, kc * 128:(kc + 1) * 128],
                                qTf[0:64, m, q0:q0 + w], start=True, stop=True,
                            )
                            nc.tensor.matmul(
                                sB[:, 0:w], kT[64:128, kc * 128:(kc + 1) * 128],
                                qTf[64:128, m, q0:q0 + w], start=True, stop=True,
                            )
                            pA = as_.tile([128, 512], BF16, tag="pA", bufs=3)
                            pB = as_.tile([128, 512], BF16, tag="pB", bufs=3)
                            nc.scalar.activation(pA[:, 0:w], sA[:, 0:w], AF.Exp, scale=EXP_SCALE)
                            nc.scalar.activation(pB[:, 0:w], sB[:, 0:w], AF.Exp, scale=EXP_SCALE)
                            if i >= 0:
                                nc.vector.tensor_mul(pA[:, 0:128], pA[:, 0:128], trimask[:])
                                nc.vector.tensor_mul(pB[:, 0:128], pB[:, 0:128], trimask[:])
                            nc.tensor.matmul(
                                oa[:, off:512], v_sb[:, kc, vslot:vslot + 65],
                                pA[:, 0:w], start=(kc == 0), stop=(kc == nkc - 1),
                                skip_group_check=True,
                            )
                            nc.tensor.matmul(
                                ob[:, off:512], v_sb[:, kc, vslot:vslot + 65],
                                pB[:, 0:w], start=(kc == 0), stop=(kc == nkc - 1),
                                skip_group_check=True,
                            )
                        # normalize + evict: r = 1/rowsum, broadcast via K=1 matmul
                        ssum = as_.tile([1, 1024], F32, tag="ssum")
                        nc.vector.tensor_copy(ssum[:, 0:512], oa[64:65, :])
                        nc.vector.tensor_copy(ssum[:, 512:1024], ob[64:65, :])
                        rrf = as_.tile([1, 1024], F32, tag="rrf")
                        nc.vector.reciprocal(rrf[:], ssum[:])
                        rrb = as_.tile([1, 1024], BF16, tag="rrb")
                        nc.vector.tensor_copy(rrb[:], rrf[:])
                        rbA = ap_.tile([64, 512], F32, tag="rbA")
                        rbB = ap_.tile([64, 512], F32, tag="rbB")
                        nc.tensor.matmul(rbA[:], expA[0:1, :], rrb[:, 0:512], start=True, stop=True)
                        nc.tensor.matmul(rbB[:], expA[0:1, :], rrb[:, 512:1024], start=True, stop=True)
                        rbAs = as_.tile([64, 512], BF16, tag="rbAs")
                        rbBs = as_.tile([64, 512], BF16, tag="rbBs")
                        nc.vector.tensor_copy(rbAs[:], rbA[:])
                        nc.vector.tensor_copy(rbBs[:], rbB[:])
                        nc.vector.tensor_mul(
                            oT[0:64, m, 512 * j:512 * (j + 1)], oa[0:64, :], rbAs[:]
                        )
                        tmpB = as_.tile([64, 512], BF16, tag="tmpB")
                        nc.vector.tensor_mul(tmpB[:], ob[0:64, :], rbBs[:])
                        nc.vector.tensor_copy(oT[64:128, m, 512 * j:512 * (j + 1)], tmpB[:])

            # ---- phase E: pair AllGather of O^T ----
            cc_in = dram.tile([128, 4 * N], BF16)
            cc_out = dram.tile([2, 128, 4 * N], BF16)
            nc.sync.dma_start(out=cc_in[:], in_=oT.rearrange("p a b -> p (a b)"))
            nc.gpsimd.collective_compute(
                "AllGather",
                mybir.AluOpType.bypass,
                replica_groups=[[0, 1], [2, 3], [4, 5], [6, 7]],
                ins=[cc_in.opt()],
                outs=[cc_out.opt()],
            )
            for r in range(2):
                nc.sync.dma_start(
                    out=oT_all[:, r * 4:(r + 1) * 4, :].rearrange("p a b -> p (a b)"),
                    in_=cc_out[r],
                )

            # ---- phase F: out projection ----
            with (
                tc.tile_pool(name="op_psum", bufs=2, space="PSUM") as op_,
                tc.tile_pool(name="op_sbuf", bufs=3) as os_,
            ):
                for tcix in range(NTC):
                    po = op_.tile([128, 512], F32, tag="po")
                    for rc in range(8):
                        nc.tensor.matmul(
                            po[:], oT_all[:, rc, tcix * 128:(tcix + 1) * 128],
                            wo_sb[:, rc, :], start=(rc == 0), stop=(rc == 7),
                        )
                    ev = os_.tile([128, 512], F32, tag="ev")
                    nc.vector.tensor_copy(ev[:], po[:])
                    nc.sync.dma_start(
                        out=out_ext[tcix * 128:(tcix + 1) * 128, :], in_=ev[:]
                    )

    nc.finalize()
    return nc


_NC_CACHE = None


def _get_nc():
    global _NC_CACHE
    if _NC_CACHE is None:
        _NC_CACHE = build()
    return _NC_CACHE


def _make_in_maps(inputs):
    x = np.asarray(inputs["x"], dtype=np.float32)
    wq = np.asarray(inputs["wq"], dtype=np.float32)
    wk = np.asarray(inputs["wk"], dtype=np.float32)
    wv = np.asarray(inputs["wv"], dtype=np.float32)
    wo = np.asarray(inputs["wo"], dtype=np.float32)
    in_maps = []
    for i in range(8):
        b, p = i // 2, i % 2
        in_maps.append({
            "x": np.ascontiguousarray(x[b]),
            "wq": np.ascontiguousarray(wq[:, p * DQ:(p + 1) * DQ]),
            "wk": np.ascontiguousarray(wk[:, p * DKV:(p + 1) * DKV]),
            "wv": np.ascontiguousarray(wv[:, p * DKV:(p + 1) * DKV]),
            "wo": np.ascontiguousarray(wo[:, p * DQ:(p + 1) * DQ]),
        })
    return in_maps


def kernel(x, wq, wk, wv, wo):
    x = np.asarray(x, dtype=np.float32)
    B = x.shape[0]
    nc = _get_nc()
    in_maps = _make_in_maps({"x": x, "wq": wq, "wk": wk, "wv": wv, "wo": wo})
    res = run_bass_kernel_spmd(nc, in_maps, core_ids=list(range(8)))
    out = np.empty((B, N, C), dtype=np.float32)
    for b in range(B):
        out[b, :, 0:DQ] = res.results[2 * b]["out"]
        out[b, :, DQ:C] = res.results[2 * b + 1]["out"]
    return out


if __name__ == "__main__":
    rng = np.random.default_rng(0)
    ins = {
        "x": rng.standard_normal((4, N, C), dtype=np.float32),
        "wq": (rng.standard_normal((C, C), dtype=np.float32) * 0.02),
        "wk": (rng.standard_normal((C, 256), dtype=np.float32) * 0.02),
        "wv": (rng.standard_normal((C, 256), dtype=np.float32) * 0.02),
        "wo": (rng.standard_normal((C, C), dtype=np.float32) * 0.02),
    }
    y = kernel(**ins)
    print("out", y.shape, y.dtype, np.abs(y).mean())
